# revision 1
# baseline (speedup 1.0000x reference)
"""Causal self-attention (B=4, T=2048, C=1024, H=16) on 8 Trainium2 cores.

Sharding: 4 pair-groups of 2 cores; group g owns batch g; within a group the
16 heads split 8+8 (tensor parallel on the head dim for qkv/out weights).
Each core computes qkv for its 8 heads, flash-style causal attention in a
"S-transposed" layout (scores kept as [key, query] so softmax denominators
ride as extra ones-columns through the PV matmul and no transposes are
needed anywhere), then its partial out-projection; a pairwise ReduceScatter
sums the two head-halves and leaves each core with half the batch's tokens.

All matmuls run in bf16 with fp32 PSUM accumulation. Softmax skips the
running-max subtraction (scores are ~N(0,1) here; exp stays well inside
fp32/bf16 range), which the reference softmax is algebraically invariant to.
"""
import numpy as np
import ml_dtypes

import concourse.bass as bass
import concourse.mybir as mybir
import concourse.tile as tile
from concourse.bass_utils import run_bass_kernel_spmd

BF16 = mybir.dt.bfloat16
F32 = mybir.dt.float32
AF = mybir.ActivationFunctionType
ALU = mybir.AluOpType

B, T, C = 4, 2048, 1024
H, DH = 16, 64
NCORES = 8
HLOC = 8            # heads per core
DLOC = HLOC * DH    # 512 local head dims
NCCH = C // 128     # 8 contraction chunks over C
NQS = T // 512      # 4 query strips
NKC = T // 128      # 16 key chunks
PAIRS = [[0, 1], [2, 3], [4, 5], [6, 7]]

_CACHE = {}


def _split_excess_waits(nc):
    """This walrus build rejects instructions carrying more than one sync
    wait; peel extras onto preceding same-engine NOPs (the engine stalls at
    each in program order, so semantics are identical)."""
    n = 0
    for bb in nc.main_func.blocks:
        new_list = []
        for ins in bb.instructions:
            w = list(ins.sync_info.on_wait) if ins.sync_info else []
            if len(w) > 1:
                for sw in w[:-1]:
                    nop = mybir.InstNoOp(
                        name=nc.get_next_instruction_name(),
                        engine=ins.engine,
                        sync_info=mybir.SyncInfo(on_wait=[sw], on_update=[]),
                    )
                    nc.register_instruction(nop)
                    new_list.append(nop)
                ins.sync_info = mybir.SyncInfo(
                    on_wait=w[-1:], on_update=list(ins.sync_info.on_update)
                )
                n += 1
            new_list.append(ins)
        bb.instructions[:] = new_list
    return n


def _build_program(do_qkv=True, do_attn=True, do_out=True, rs_mode="single", repeat=1, pz_pool="po"):
    nc = bass.Bass("TRN2", target_bir_lowering=False, debug=False, num_devices=NCORES)

    xT = nc.declare_dram_parameter("xT", [C, T], BF16, isOutput=False)
    wqT = nc.declare_dram_parameter("wqT", [C, DLOC], BF16, isOutput=False)
    wkT = nc.declare_dram_parameter("wkT", [C, DLOC], BF16, isOutput=False)
    wvT = nc.declare_dram_parameter("wvT", [C, DLOC], BF16, isOutput=False)
    owT = nc.declare_dram_parameter("owT", [DLOC, C], BF16, isOutput=False)
    bq = nc.declare_dram_parameter("bq", [128, 4], F32, isOutput=False)
    bk = nc.declare_dram_parameter("bk", [128, 4], F32, isOutput=False)
    bvb = nc.declare_dram_parameter("bvb", [128, DLOC], F32, isOutput=False)
    obb = nc.declare_dram_parameter("obb", [128, C], F32, isOutput=False)
    mask = nc.declare_dram_parameter("mask", [128, 128], BF16, isOutput=False)
    z = nc.declare_dram_parameter("z", [T // 2, C], F32, isOutput=True)

    with tile.TileContext(nc) as tc:
        with (
            tc.tile_pool(name="const", bufs=1) as const,
            tc.tile_pool(name="pers", bufs=1) as pers,
            tc.tile_pool(name="es", bufs=4) as es_pool,
            tc.tile_pool(name="osb", bufs=4) as osb_pool,
            tc.tile_pool(name="small", bufs=8) as small,
            tc.tile_pool(name="zsb", bufs=2) as zsb_pool,
            tc.tile_pool(name="dram", bufs=1, space="DRAM") as dram,
        ):
            # ---- input loads ----
            xT_sb, wq_sb, wk_sb, wv_sb = [], [], [], []
            for kc in range(NCCH):
                t = const.tile([128, T], BF16, tag=f"xT{kc}", name=f"xTs{kc}")
                nc.sync.dma_start(out=t, in_=xT[128 * kc:128 * kc + 128, :])
                xT_sb.append(t)
            for kc in range(NCCH):
                tq = const.tile([128, DLOC], BF16, tag=f"wq{kc}", name=f"wqs{kc}")
                nc.sync.dma_start(out=tq, in_=wqT[128 * kc:128 * kc + 128, :])
                wq_sb.append(tq)
                tk_ = const.tile([128, DLOC], BF16, tag=f"wk{kc}", name=f"wks{kc}")
                nc.sync.dma_start(out=tk_, in_=wkT[128 * kc:128 * kc + 128, :])
                wk_sb.append(tk_)
                tv = const.tile([128, DLOC], BF16, tag=f"wv{kc}", name=f"wvs{kc}")
                nc.sync.dma_start(out=tv, in_=wvT[128 * kc:128 * kc + 128, :])
                wv_sb.append(tv)
            ow_sb = []
            for hp in range(4):
                t = const.tile([128, C], BF16, tag=f"ow{hp}", name=f"ows{hp}")
                nc.sync.dma_start(out=t, in_=owT[128 * hp:128 * hp + 128, :])
                ow_sb.append(t)
            mask_sb = const.tile([128, 128], BF16, tag="mask")
            nc.sync.dma_start(out=mask_sb, in_=mask[:])
            bq_sb = const.tile([128, 4], F32, tag="bq")
            nc.sync.dma_start(out=bq_sb, in_=bq[:])
            bk_sb = const.tile([128, 4], F32, tag="bk")
            nc.sync.dma_start(out=bk_sb, in_=bk[:])
            bvb_sb = const.tile([128, DLOC], F32, tag="bvb")
            nc.sync.dma_start(out=bvb_sb, in_=bvb[:])
            obb_sb = const.tile([128, C], F32, tag="obb")
            nc.sync.dma_start(out=obb_sb, in_=obb[:])

            # ---- persistent intermediate tiles ----
            qT_sb = [pers.tile([128, T], BF16, tag=f"qT{i}", name=f"qT{i}") for i in range(4)]
            kT_sb = [pers.tile([128, T], BF16, tag=f"kT{i}", name=f"kT{i}") for i in range(4)]
            vaug = [pers.tile([128, HLOC, 128], BF16, tag=f"vaug{i}", name=f"vaug{i}") for i in range(NKC)]
            yT_sb = [pers.tile([128, T], BF16, tag=f"yT{i}", name=f"yT{i}") for i in range(4)]
            for i in range(NKC):
                nc.vector.memset(vaug[i][:, :, 64:128], 1.0)
            if not do_qkv:  # phase-probe stubs
                for i in range(4):
                    nc.vector.memset(qT_sb[i], 0.0)
                    nc.vector.memset(kT_sb[i], 0.0)
                for i in range(NKC):
                    nc.vector.memset(vaug[i][:, :, 0:64], 0.0)
            if not do_attn:
                for i in range(4):
                    nc.vector.memset(yT_sb[i], 0.0)

            # ---- DRAM bounce buffers for the pairwise reduce-scatter ----
            zpart_full = dram.tile([T, C], F32, tag="zpartf", name="zpartf")
            zshard_full = dram.tile([T // 2, C], F32, tag="zshardf", name="zshardf")
            zpart = [zpart_full[512 * b:512 * b + 512, :] for b in range(4)]
            zshard = [dram.tile([256, C], F32, tag=f"zshard{b}", name=f"zshard{b}") for b in range(4)]

            # ---- phases B..D: qkv interleaved into attention; out-proj per strip ----
            for _rep in range(repeat):
              with (
                tc.tile_pool(name="ps_qkv", bufs=2, space="PSUM") as ps_qkv,
                tc.tile_pool(name="ps_s", bufs=2, space="PSUM") as ps_s,
                tc.tile_pool(name="ps_o", bufs=2, space="PSUM") as ps_o,
              ):

                def qk_group(ci, ts, w_sb, b_sb, dst):
                    p = ps_qkv.tile([128, 512], F32, tag="pqkv", name="pqk")
                    for kc in range(NCCH):
                        nc.tensor.matmul(
                            p,
                            lhsT=w_sb[kc][:, 128 * ci:128 * ci + 128],
                            rhs=xT_sb[kc][:, 512 * ts:512 * ts + 512],
                            start=(kc == 0),
                            stop=(kc == NCCH - 1),
                        )
                    nc.vector.tensor_scalar(
                        out=dst[ci][:, 512 * ts:512 * ts + 512],
                        in0=p,
                        scalar1=b_sb[:, ci:ci + 1],
                        scalar2=None,
                        op0=ALU.add,
                    )

                def v_group(tc2):
                    pv = ps_qkv.tile([128, 512], F32, tag="pqkv", name="pv")
                    for kc in range(NCCH):
                        nc.tensor.matmul(
                            pv,
                            lhsT=xT_sb[kc][:, 128 * tc2:128 * tc2 + 128],
                            rhs=wv_sb[kc],
                            start=(kc == 0),
                            stop=(kc == NCCH - 1),
                        )
                    nc.vector.tensor_tensor(
                        out=vaug[tc2][:, :, 0:64],
                        in0=pv.rearrange("p (h d) -> p h d", h=HLOC),
                        in1=bvb_sb.rearrange("p (h d) -> p h d", h=HLOC),
                        op=ALU.add,
                    )

                def unit_thunks(ts):
                    th = []
                    for ci in range(4):
                        th.append(lambda ci=ci, ts=ts: qk_group(ci, ts, wq_sb, bq_sb, qT_sb))
                        th.append(lambda ci=ci, ts=ts: qk_group(ci, ts, wk_sb, bk_sb, kT_sb))
                    for tc2 in range(4 * ts, 4 * ts + 4):
                        th.append(lambda tc2=tc2: v_group(tc2))
                    return th

                if do_qkv:
                    for th in unit_thunks(0):
                        th()

                def out_block_thunks(blk):
                    th = []
                    for tc2 in range(4):
                        def one(tc2=tc2, blk=blk):
                            t0 = 512 * blk + 128 * tc2
                            zb = zsb_pool.tile([128, C], F32, tag="zsb", name="zb")
                            for zc in range(2):
                                pz = (ps_qkv.tile([128, 512], F32, tag="pqkv", name="pz") if pz_pool == "qkv" else ps_o.tile([128, 512], F32, tag="po", name="pz"))
                                for hp in range(4):
                                    nc.tensor.matmul(
                                        pz,
                                        lhsT=yT_sb[hp][:, t0:t0 + 128],
                                        rhs=ow_sb[hp][:, 512 * zc:512 * zc + 512],
                                        start=(hp == 0),
                                        stop=(hp == 3),
                                    )
                                nc.vector.tensor_add(
                                    out=zb[:, 512 * zc:512 * zc + 512],
                                    in0=pz,
                                    in1=obb_sb[:, 512 * zc:512 * zc + 512],
                                )
                            nc.sync.dma_start(
                                out=zpart[blk][128 * tc2:128 * tc2 + 128, :], in_=zb
                            )
                        th.append(one)
                    return th

                deferred_out = []
                for j in range(NQS):
                    nch = 4 * (j + 1)
                    # filler work spliced between attention chunk units so PE
                    # has dense work while ACT runs the exps: strips 0-2 carry
                    # the next strip's qkv; the last strip carries the
                    # deferred out-projection blocks.
                    if do_qkv and j < NQS - 1:
                        pending = unit_thunks(j + 1)
                    elif j == NQS - 1:
                        pending = deferred_out
                    else:
                        pending = []
                    n_pend = len(pending)
                    stride = max(1, (4 * nch) // max(1, n_pend))
                    state = {"u": 0}

                    def tick(pending=pending, n_pend=n_pend, stride=stride, state=state):
                        state["u"] += 1
                        while pending and state["u"] >= stride * (n_pend - len(pending) + 1):
                            pending.pop(0)()

                    if do_attn:
                        for hp in range(4):
                            po = [
                                ps_o.tile([128, 512], F32, tag="po", name="po0"),
                                ps_o.tile([128, 512], F32, tag="po", name="po1"),
                            ]
                            es_tiles = {}

                            def emit_mm1(ck, j=j, hp=hp, es_tiles=es_tiles):
                                # on diagonal chunks only queries q >= z0 can
                                # attend this chunk's keys; compute just that
                                # column range end-to-end (MM1, exp, MM2).
                                r_off = ck - 4 * j
                                z0 = 128 * r_off if r_off >= 0 else 0
                                pS = ps_s.tile([128, 1024], F32, tag="pS", name="pS")
                                pS3 = pS.rearrange("p (h q) -> p h q", h=2)
                                for h in range(2):
                                    nc.tensor.matmul(
                                        pS3[:, h, z0:512],
                                        lhsT=kT_sb[hp][64 * h:64 * h + 64, 128 * ck:128 * ck + 128],
                                        rhs=qT_sb[hp][64 * h:64 * h + 64, 512 * j + z0:512 * j + 512],
                                        start=True,
                                        stop=True,
                                    )
                                eS = es_pool.tile([128, 2, 512], BF16, tag="eS", name="eS")
                                nc.scalar.activation(
                                    out=eS[:, :, z0:512],
                                    in_=pS3[:, :, z0:512],
                                    func=AF.Exp,
                                    scale=0.125,
                                )
                                if r_off >= 0:
                                    for h in range(2):
                                        nc.vector.tensor_mul(
                                            out=eS[:, h, z0:z0 + 128],
                                            in0=eS[:, h, z0:z0 + 128],
                                            in1=mask_sb,
                                        )
                                es_tiles[ck] = (eS, z0)

                            def emit_mm2(ck, hp=hp, nch=nch, po=po, es_tiles=es_tiles):
                                eS, z0 = es_tiles.pop(ck)
                                for h in range(2):
                                    nc.tensor.matmul(
                                        po[h][:, z0:512],
                                        lhsT=vaug[ck][:, 2 * hp + h, :],
                                        rhs=eS[:, h, z0:512],
                                        start=(ck == 0),
                                        stop=(ck == nch - 1),
                                        skip_group_check=True,
                                    )

                            emit_mm1(0)
                            for ck in range(1, nch):
                                emit_mm1(ck)
                                emit_mm2(ck - 1)
                                tick()
                            emit_mm2(nch - 1)
                            tick()

                            # move O-accum out of PSUM fast, then normalize.
                            # po rows 0:64 hold unnormalized O^T, rows 64:128
                            # hold 64 replicated copies of sum(exp).
                            for h in range(2):
                                ot = osb_pool.tile([128, 512], F32, tag="osb", name="ot")
                                nc.vector.tensor_copy(out=ot, in_=po[h])
                                rcp = small.tile([64, 512], F32, tag="rcp", name="rcp")
                                nc.vector.reciprocal(out=rcp, in_=ot[64:128, :])
                                nc.vector.tensor_mul(
                                    out=yT_sb[hp][64 * h:64 * h + 64, 512 * j:512 * j + 512],
                                    in0=ot[0:64, :],
                                    in1=rcp,
                                )

                    # drain any filler thunks the tick schedule didn't reach
                    while pending:
                        pending.pop(0)()

                    # out-projection for this 512-token block + reduce-scatter
                    if do_out:
                        blk = j
                        if rs_mode == "single" and j < NQS - 1:
                            deferred_out.extend(out_block_thunks(blk))
                            continue
                        for th in out_block_thunks(blk):
                            th()
                        if rs_mode == "block":
                            nc.gpsimd.collective_compute(
                                "ReduceScatter",
                                ALU.add,
                                replica_groups=PAIRS,
                                ins=[zpart[blk].opt()],
                                outs=[zshard[blk][:].opt()],
                            )
                            nc.sync.dma_start(
                                out=z[256 * blk:256 * blk + 256, :], in_=zshard[blk][:]
                            )
                        elif rs_mode == "none":
                            nc.sync.dma_start(
                                out=z[256 * blk:256 * blk + 256, :],
                                in_=zpart[blk][0:256, :],
                            )
                if do_out and rs_mode == "single":
                    nc.gpsimd.collective_compute(
                        "ReduceScatter",
                        ALU.add,
                        replica_groups=PAIRS,
                        ins=[zpart_full[:].opt()],
                        outs=[zshard_full[:].opt()],
                    )
                    nc.sync.dma_start(out=z[:], in_=zshard_full[:])

    _split_excess_waits(nc)
    return nc


def _get_program():
    if "nc" not in _CACHE:
        _CACHE["nc"] = _build_program()
    return _CACHE["nc"]


def make_in_maps(x, qkv_w, qkv_b, out_w, out_b):
    bf = ml_dtypes.bfloat16
    x = np.asarray(x, dtype=np.float32)
    qkv_w = np.asarray(qkv_w, dtype=np.float32)
    qkv_b = np.asarray(qkv_b, dtype=np.float32)
    out_w = np.asarray(out_w, dtype=np.float32)
    out_b = np.asarray(out_b, dtype=np.float32)

    mask_np = (np.arange(128)[:, None] <= np.arange(128)[None, :]).astype(bf)
    obb_np = np.ascontiguousarray(
        np.broadcast_to(out_b / 2.0, (128, C)).astype(np.float32)
    )
    in_maps = []
    for c in range(NCORES):
        g, r = divmod(c, 2)
        sl = slice(r * DLOC, (r + 1) * DLOC)
        in_maps.append(
            dict(
                xT=np.ascontiguousarray(x[g].T).astype(bf),
                wqT=np.ascontiguousarray(qkv_w[0 * C:1 * C][sl].T).astype(bf),
                wkT=np.ascontiguousarray(qkv_w[1 * C:2 * C][sl].T).astype(bf),
                wvT=np.ascontiguousarray(qkv_w[2 * C:3 * C][sl].T).astype(bf),
                owT=np.ascontiguousarray(out_w[:, sl].T).astype(bf),
                bq=np.ascontiguousarray(qkv_b[0 * C:1 * C][sl].reshape(4, 128).T).astype(np.float32),
                bk=np.ascontiguousarray(qkv_b[1 * C:2 * C][sl].reshape(4, 128).T).astype(np.float32),
                bvb=np.ascontiguousarray(
                    np.broadcast_to(qkv_b[2 * C:3 * C][sl], (128, DLOC))
                ).astype(np.float32),
                obb=obb_np,
                mask=mask_np,
            )
        )
    return in_maps


def assemble_output(results):
    # rs_mode="single": rank r of pair g holds tokens [1024r, 1024r+1024)
    out = np.empty((B, T, C), np.float32)
    for g in range(4):
        for r in range(2):
            out[g, 1024 * r:1024 * r + 1024, :] = results[2 * g + r]["z"]
    return out


def kernel(x, qkv_w, qkv_b, out_w, out_b):
    nc = _get_program()
    in_maps = make_in_maps(x, qkv_w, qkv_b, out_w, out_b)
    res = run_bass_kernel_spmd(nc, in_maps, list(range(NCORES)))
    return assemble_output(res.results)

